# revision 29
# baseline (speedup 1.0000x reference)
"""Multi-head attention (QKV proj + RoPE + causal SDPA + out proj) on 8 TRN2 cores.

Sharding: core c = 4*b + g handles batch b (of 2) and head-group g (of 4, i.e.
4 heads = 512 feature dims). Per-core device kernel computes, for its batch:
    q/k/v projections for its 4 heads (column-sharded w_q / w_kv),
    RoPE on q and k, causal softmax attention,
    partial output projection with its 512 rows of w_o (+ bias on g==0 cores).
Host gathers by summing the 4 per-group partials per batch (the "all-reduce"
for the row-sharded w_o, done as the unshard step).

All inputs are cast to bf16 on the host so DMAs land directly in their final
SBUF tiles (no staging copies / on-device casts). Output is bf16, upcast and
summed on the host.

Device dataflow is feature-major: the host passes x pre-transposed (xT[e, s]);
projections produce qT/kT in [head_dim, seq] layout and v in [seq, head_dim]
layout, which is exactly what the S^T = K.Q^T and O^T = V^T.P^T matmuls need.

Engine balance: PE does only matmuls; ACT does the softmax exp (and the
PSUM->SBUF copies feeding RoPE / v, emitted during projection rounds where it
is otherwise idle); DVE does RoPE arithmetic, the P-accumulation that replaces
per-block rowsum matmuls, and PSUM evacuation; GpSimd broadcasts 1/rowsum.
Because exp throughput (1.2 G elem/s/lane) is slightly below the 2-matmul PE
pace, independent matmul chains (next chunk's q/k projections, then the output
projection) are interleaved into the attention instruction stream so the PE
never waits on ACT.
"""

import sys

import numpy as np

sys.path.insert(0, "/opt/trn_rl_repo")

EMB = 2048
SEQ = 2048
N_HEAD = 16
HD = 128
BATCH = 2
N_CORES = 8
GROUPS = 4  # head groups (tensor-parallel dimension)
HPG = N_HEAD // GROUPS  # heads per group = 4
DPG = HPG * HD  # feature dims per group = 512
NE = EMB // 128  # 16 e-blocks
SCALE = float(HD) ** -0.5


def _host_tables(seq):
    """cos / sign-folded sin RoPE tables in [d, s] layout + triangle mask."""
    d = HD
    inv = 1.0 / (10000.0 ** (np.arange(0, d, 2, dtype=np.float64) / d))  # [64]
    pos = np.arange(seq, dtype=np.float64)[None, :] * inv[:, None]  # [64, s]
    ang = np.concatenate([pos, pos], axis=0)  # [128, s]
    cos_t = np.cos(ang)
    sin = np.sin(ang)
    # rot is built as a plain partition swap (rot[0:64]=q[64:128], rot[64:128]=q[0:64]);
    # the rotate_half sign lives in the sin table instead.
    sinm = np.concatenate([-sin[:64], sin[64:]], axis=0)
    # triangle mask for the diagonal 128x128 block: keep (ko, qo) iff qo >= ko
    ko = np.arange(128)[:, None]
    qo = np.arange(128)[None, :]
    mask_t = (qo >= ko).astype(np.float64)
    return cos_t, sinm, mask_t


def build(seq=SEQ, has_bias=False):
    """Build the per-core Bass program. Returns the compiled Bacc module."""
    import concourse.bacc as bacc
    import concourse.tile as tile
    from concourse import mybir

    bf16 = mybir.dt.bfloat16

    assert seq % 512 == 0

    nc = bacc.Bacc("TRN2", target_bir_lowering=False, debug=False,
                   num_devices=N_CORES, name="mha8")

    # host pre-arranges x / weights so every DMA lands per-partition
    # contiguous (128 descriptors x 16KB instead of 1024 x 1KB)
    nj_ = seq // 512
    xt_d = nc.dram_tensor("xt", [nj_ * 128, NE * 512], bf16,
                          kind="ExternalInput")
    wq_d = nc.dram_tensor("wq", [128, NE * DPG], bf16, kind="ExternalInput")
    wk_d = nc.dram_tensor("wk", [128, NE * DPG], bf16, kind="ExternalInput")
    wv_d = nc.dram_tensor("wv", [128, NE * DPG], bf16, kind="ExternalInput")
    wo_d = nc.dram_tensor("wo", [128, HPG * EMB], bf16, kind="ExternalInput")
    bo_d = nc.dram_tensor("bo", [1, EMB], bf16, kind="ExternalInput")
    cos_d = nc.dram_tensor("cosT", [HD, seq], bf16, kind="ExternalInput")
    sinm_d = nc.dram_tensor("sinM", [HD, seq], bf16, kind="ExternalInput")
    mask_d = nc.dram_tensor("maskT", [128, 128], bf16, kind="ExternalInput")
    out_d = nc.dram_tensor("out", [seq, EMB], bf16, kind="ExternalOutput")

    with tile.TileContext(nc) as tc:
        _emit(nc, tc, tile, mybir, seq,
              xt_d, wq_d, wk_d, wv_d, wo_d, bo_d, cos_d, sinm_d, mask_d, out_d,
              has_bias)
    nc.compile()
    return nc


def _emit(nc, tc, tile, mybir, seq,
          xt_d, wq_d, wk_d, wv_d, wo_d, bo_d, cos_d, sinm_d, mask_d, out_d,
          has_bias):
    from contextlib import ExitStack

    f32 = mybir.dt.float32
    bf16 = mybir.dt.bfloat16
    EXP = mybir.ActivationFunctionType.Exp
    COPY = mybir.ActivationFunctionType.Copy
    nj = seq // 512  # 512-wide q chunks
    nsb = seq // 128

    ctx = ExitStack()
    with ctx:
        persist = ctx.enter_context(tc.tile_pool(name="persist", bufs=1))
        wpool = ctx.enter_context(tc.tile_pool(name="wpool", bufs=1, side="right"))

        # ---- constants / persistent tiles ----
        # all-ones stationary: rowsum matmul ones_mat.T @ acc yields the
        # rowsum already replicated across all 128 partitions
        ones_mat = persist.tile([128, 128], bf16, name="ones_mat")
        nc.vector.memset(ones_mat, 1.0)
        dummy = persist.tile([1, 1], f32, name="dummy")
        nc.vector.memset(dummy, 0.0)
        # pre-trigger the exp ACT table load so it overlaps the input DMAs
        nc.scalar.activation(dummy, dummy, EXP)

        mask_sb = persist.tile([128, 128], bf16, name="mask_sb")
        cos_sb = wpool.tile([128, seq], bf16, name="cos_sb")
        sinm_sb = wpool.tile([128, seq], bf16, name="sinm_sb")

        # per-head tensors (separate tiles so cross-head readers/writers never
        # serialize on conservative whole-tile dependencies)
        kt = [persist.tile([128, seq], bf16, name=f"kt{h}") for h in range(HPG)]
        yt = [persist.tile([128, seq], bf16, name=f"yt{h}") for h in range(HPG)]
        v_sb = persist.tile([128, nsb, DPG], bf16, name="v_sb")  # [s_in, blk, d]

        w_sb = {nm: wpool.tile([128, NE, DPG], bf16, name=f"{nm}_sb")
                for nm in ("wq", "wk", "wv")}
        wo_sb = wpool.tile([128, HPG, EMB], bf16, name="wo_sb")

        xt_pool = ctx.enter_context(tc.tile_pool(name="xt", bufs=2, side="right"))
        qtj_pool = ctx.enter_context(tc.tile_pool(name="qtj", bufs=2))
        rope_pool = ctx.enter_context(tc.tile_pool(name="rope", bufs=2))
        pt_pool = ctx.enter_context(tc.tile_pool(name="pt", bufs=4))
        acc_pool = ctx.enter_context(tc.tile_pool(name="accp", bufs=2))
        sm_pool = ctx.enter_context(tc.tile_pool(name="sm", bufs=2))
        ob_pool = ctx.enter_context(tc.tile_pool(name="obp", bufs=2))

        st_pool = ctx.enter_context(tc.tile_pool(name="stp", bufs=3, space="PSUM"))
        ot_pool = ctx.enter_context(tc.tile_pool(name="otp", bufs=2, space="PSUM"))
        rs_pool = ctx.enter_context(tc.tile_pool(name="rsp", bufs=1, space="PSUM"))
        ps1_ctx = ExitStack()
        ps1 = ps1_ctx.enter_context(tc.tile_pool(name="ps1", bufs=2, space="PSUM"))

        # ---- DMA helpers (all direct bf16, no staging) ----
        def load_w(nm, wd, engs=(None, None), parts=None):
            # e-chunks; each stays a contiguous per-partition run. More chunks
            # -> finer completion semaphores (consumers start sooner).
            n = len(parts or engs)
            epp = NE // n
            for q in range(n):
                src = wd[:, q * epp * DPG:(q + 1) * epp * DPG].rearrange(
                    "p (e d) -> p e d", e=epp)
                eng = (parts or engs)[q] or nc.sync
                eng.dma_start(w_sb[nm][:, q * epp:(q + 1) * epp, :], src)

        def load_xt(j, engs=(None, None), parts=None):
            xt_j = xt_pool.tile([128, NE, 512], bf16, name=f"xt_{j}", tag="xt")
            rows = xt_d[j * 128:(j + 1) * 128, :]
            n = len(parts or engs)
            epp = NE // n
            for q in range(n):
                src = rows[:, q * epp * 512:(q + 1) * epp * 512].rearrange(
                    "p (e s) -> p e s", e=epp)
                eng = (parts or engs)[q] or nc.sync
                eng.dma_start(xt_j[:, q * epp:(q + 1) * epp, :], src)
            return xt_j

        def load_wo():
            for half in range(2):
                src = wo_d[:, half * 2 * EMB:(half + 1) * 2 * EMB].rearrange(
                    "p (h e) -> p h e", h=2)
                eng = nc.scalar if half else nc.sync
                eng.dma_start(wo_sb[:, half * 2:(half + 1) * 2, :], src)

        # ---- compute helpers ----
        def rope(dst, h, j, pp, tag):
            """dst = rope(pp) (chunk j); pp is the f32 PSUM projection."""
            sl = slice(j * 512, (j + 1) * 512)
            # ACT evacuates PSUM (idle during projection work); DVE does the rest
            qs = rope_pool.tile([128, 512], bf16, name=f"qs_{tag}", tag="qs")
            nc.scalar.activation(qs, pp, COPY)
            rot = rope_pool.tile([128, 512], bf16, name=f"rot_{tag}", tag="rot")
            # rotate_half as partition-shifted copies (sign folded into sinM);
            # TensorTensor ops must be partition-aligned, plain copies may shift
            nc.vector.tensor_copy(rot[0:64, :], qs[64:128, :])
            nc.vector.tensor_copy(rot[64:128, :], qs[0:64, :])
            nc.vector.tensor_mul(rot, rot, sinm_sb[:, sl])   # in-place
            nc.vector.tensor_mul(qs, qs, cos_sb[:, sl])      # in-place
            nc.vector.tensor_add(dst, qs, rot)

        qt_tiles = {}

        def qk_chain(j, h, nm, xt_j):
            """One 16-matmul projection chain + rope for (chunk j, head h)."""
            if j not in qt_tiles:
                qt_tiles[j] = qtj_pool.tile([128, HPG, 512], bf16,
                                            name=f"qt_{j}", tag="qtj")
            pp = ps1.tile([128, 512], f32, name=f"pp_{nm}_{h}_{j}", tag="proj")
            for e in range(NE):
                nc.tensor.matmul(pp, w_sb[nm][:, e, h * 128:(h + 1) * 128],
                                 xt_j[:, e, :], start=(e == 0), stop=(e == NE - 1))
            if nm == "wq":
                rope(qt_tiles[j][:, h, :], h, j, pp, f"q{h}_{j}")
            else:
                rope(kt[h][:, j * 512:(j + 1) * 512], h, j, pp, f"k{h}_{j}")

        def v_chain(j, sb, xt_j):
            i_blk = j * 4 + sb
            vp = ps1.tile([128, DPG], f32, name=f"vp_{i_blk}", tag="proj")
            for e in range(NE):
                nc.tensor.matmul(vp, xt_j[:, e, sb * 128:(sb + 1) * 128],
                                 w_sb["wv"][:, e, :], start=(e == 0),
                                 stop=(e == NE - 1))
            nc.scalar.activation(v_sb[:, i_blk, :], vp, COPY)

        bo_sb = ones_row = None

        def op_block(sb, pool, out_eng, cast_eng=None, split_dma=False):
            """Output projection for seq row-block sb: [128, 2048] partial."""
            ssl = slice(sb * 128, (sb + 1) * 128)
            ob = ob_pool.tile([128, EMB], bf16, name=f"ob_{sb}", tag="ob")
            for ec in range(EMB // 512):
                esl = slice(ec * 512, (ec + 1) * 512)
                op = pool.tile([128, 512], f32, name=f"op_{sb}_{ec}", tag="proj")
                for h in range(HPG):
                    nc.tensor.matmul(op, yt[h][:, ssl], wo_sb[:, h, esl],
                                     start=(h == 0),
                                     stop=(not has_bias and h == HPG - 1))
                if has_bias:
                    nc.tensor.matmul(op, ones_row, bo_sb[:, esl],
                                     start=False, stop=True)
                if cast_eng is nc.scalar:
                    nc.scalar.copy(ob[:, esl], op)
                else:
                    nc.vector.tensor_copy(ob[:, esl], op)
                if split_dma and ec % 2:  # drain each half as soon as cast
                    out_eng.dma_start(out_d[ssl, (ec - 1) * 512:(ec + 1) * 512],
                                      ob[:, (ec - 1) * 512:(ec + 1) * 512])
            # during attention the scalar/ACT ring must stay free for exps (a
            # waiting DMA at the ACT queue head would block them) -> sync only
            if not split_dma:
                out_eng.dma_start(out_d[ssl, :], ob)

        # ---- preamble DMAs ----
        # wq on the scalar ring / xt0 on the sync ring, both in e-quarters so
        # the first projection chain starts on quarter 0 (~10us) instead of
        # waiting for the full tensors; tables follow, then wk, wv.
        load_w("wq", wq_d, parts=[nc.scalar] * 4)
        xt_cur = load_xt(0, parts=[nc.sync] * 4)
        nc.sync.dma_start(cos_sb, cos_d[:])
        nc.sync.dma_start(sinm_sb, sinm_d[:])
        nc.scalar.dma_start(mask_sb, mask_d[:])
        load_w("wk", wk_d, (nc.scalar, nc.sync))
        load_w("wv", wv_d, (nc.scalar, nc.sync))
        if has_bias:
            ones_row = persist.tile([1, 128], bf16, name="ones_row")
            nc.vector.memset(ones_row, 1.0)
            bo_sb = persist.tile([1, EMB], bf16, name="bo_sb")
            nc.scalar.dma_start(bo_sb, bo_d[:])

        # round 0 projections run before any attention exists to interleave;
        # q chains first (wq arrives before wk)
        for nm in ("wq", "wk"):
            for h in range(HPG):
                qk_chain(0, h, nm, xt_cur)

        xt_next = load_xt(1)
        load_wo()

        # ---- main rounds ----
        for j in range(nj):
            for sb in range(4):
                v_chain(j, sb, xt_cur)

            if j == nj - 1:
                # projections done: free ps1's banks and reopen them for the
                # interleaved output projection
                ps1_ctx.close()
                ps3 = ctx.enter_context(tc.tile_pool(name="ps3", bufs=2,
                                                     space="PSUM"))

            # filler queue: independent PE chains interleaved into attention
            # so the PE keeps running while ACT works through the exps
            fillers = []
            if j + 1 < nj:
                xt_for_next = xt_next
                for h in range(HPG):
                    for nm in ("wq", "wk"):
                        fillers.append((lambda jj=j + 1, hh=h, nn=nm,
                                        xx=xt_for_next:
                                        qk_chain(jj, hh, nn, xx)))
            else:
                for sb in range(12):
                    fillers.append(lambda s=sb: op_block(s, ps3, nc.sync))
            fill_idx = 0
            # fillers per head, always one at the head's LAST block (the
            # S->exp->PV drain there otherwise idles the PE ~1us), rest spread
            fph = max(1, len(fillers) // HPG)
            nblk = 4 * j + 4
            fill_pos = set()
            for t in range(fph):
                fill_pos.add(nblk - 1 if t == fph - 1
                             else ((t + 1) * nblk) // fph - 1)

            def make_norm(h, acc, ot):
                def norm():
                    # one matmul computes the rowsum AND broadcasts it across
                    # partitions (all-ones stationary); recip runs 128-lane
                    rb = rs_pool.tile([128, 512], f32, name=f"rb_{h}_{j}",
                                      tag="rs")
                    nc.tensor.matmul(rb, ones_mat, acc, start=True, stop=True)
                    rbf = sm_pool.tile([128, 512], f32, name=f"rbf_{h}_{j}",
                                       tag="rbf")
                    nc.vector.reciprocal_approx_fast(rbf, rb)
                    nc.vector.tensor_mul(yt[h][:, j * 512:(j + 1) * 512],
                                         ot, rbf)
                return norm

            pending_norm = None
            for h in range(HPG):
                ot = ot_pool.tile([128, 512], f32, name=f"ot_{h}_{j}", tag="ot")
                acc = acc_pool.tile([128, 512], bf16, name=f"acc_{h}_{j}",
                                    tag="acc")
                for i in range(nblk):
                    m = i - 4 * j  # diagonal index (>=0 on the 4 trailing blocks)
                    qoff = max(m, 0) * 128
                    n = 512 - qoff
                    st = st_pool.tile([128, 512], f32, name=f"st_{h}_{j}_{i}",
                                      tag="st", bufs=3)
                    nc.tensor.matmul(
                        st[:, 0:n], kt[h][:, i * 128:(i + 1) * 128],
                        qt_tiles[j][:, h, qoff:512], start=True, stop=True)
                    pt = pt_pool.tile([128, 512], bf16, name=f"pt_{h}_{j}_{i}",
                                      tag="pt")
                    nc.scalar.activation(pt[:, 0:n], st[:, 0:n], EXP, scale=SCALE)
                    # previous head's normalize slots in here: its rowsum
                    # matmul rides behind this block's S while the exp runs
                    if pending_norm is not None:
                        pending_norm()
                        pending_norm = None
                    # filler sits between this block's S and PV: the PE chews
                    # through it while ACT finishes the exp, so PV never waits
                    if i in fill_pos and fill_idx < len(fillers):
                        fillers[fill_idx]()
                        fill_idx += 1
                    if m >= 0:  # triangle mask on the leading 128 valid q cols
                        nc.vector.tensor_mul(pt[:, 0:128], pt[:, 0:128], mask_sb)
                    if i == 0:
                        nc.vector.tensor_copy(acc, pt)
                    else:
                        nc.vector.tensor_add(acc[:, qoff:512], acc[:, qoff:512],
                                             pt[:, 0:n])
                    nc.tensor.matmul(ot[:, qoff:512],
                                     v_sb[:, i, h * 128:(h + 1) * 128],
                                     pt[:, 0:n],
                                     start=(i == 0), stop=(i == nblk - 1))
                pending_norm = make_norm(h, acc, ot)
            pending_norm()
            while fill_idx < len(fillers):
                fillers[fill_idx]()
                fill_idx += 1

            xt_cur = xt_next
            if j + 2 < nj:
                xt_next = load_xt(j + 2)

        # remaining output-projection row blocks (need round-3 attention);
        # attention is over so the scalar ring/engine are free again
        for sb in range(12, nsb):
            op_block(sb, ps3, nc.scalar if sb % 2 else nc.sync,
                     cast_eng=nc.scalar, split_dma=True)


_NC_CACHE = {}


def _get_nc(seq=SEQ, has_bias=False):
    key = (seq, has_bias)
    if key not in _NC_CACHE:
        _NC_CACHE[key] = build(seq, has_bias)
    return _NC_CACHE[key]


def make_in_maps(x, w_kv, w_q, w_o, b_o, seq=SEQ):
    """Shard full inputs into the 8 per-core input dicts (all bf16)."""
    import ml_dtypes

    bf = ml_dtypes.bfloat16
    cos_t, sinm, mask_t = _host_tables(seq)
    cos_t = np.ascontiguousarray(cos_t.astype(bf))
    sinm = np.ascontiguousarray(sinm.astype(bf))
    mask_t = np.ascontiguousarray(mask_t.astype(bf))
    zeros_bo = np.zeros((1, EMB), bf)
    x = np.asarray(x, np.float32)
    w_kv = np.asarray(w_kv, np.float32)
    w_q = np.asarray(w_q, np.float32)
    w_o = np.asarray(w_o, np.float32)
    b_o = np.asarray(b_o, np.float32)
    nj = seq // 512

    def arr_w(w):  # [2048, 512] -> [128, 16*512]: row p holds (e, d) runs
        return np.ascontiguousarray(
            w.reshape(NE, 128, DPG).transpose(1, 0, 2).reshape(128, NE * DPG)
            .astype(bf))

    def arr_wo(w):  # [512, 2048] -> [128, 4*2048]
        return np.ascontiguousarray(
            w.reshape(HPG, 128, EMB).transpose(1, 0, 2).reshape(128, HPG * EMB)
            .astype(bf))

    def arr_xt(xb):  # [seq, 2048] -> [nj*128, 16*512]: xT chunked + contiguous
        a = xb.reshape(nj, 512, NE, 128).transpose(0, 3, 2, 1)
        return np.ascontiguousarray(
            a.reshape(nj * 128, NE * 512).astype(bf))

    xts = [arr_xt(x[b]) for b in range(BATCH)]
    in_maps = []
    for c in range(N_CORES):
        b, g = divmod(c, GROUPS)
        d0 = g * DPG
        in_maps.append({
            "xt": xts[b],
            "wq": arr_w(w_q[:, d0:d0 + DPG]),
            "wk": arr_w(w_kv[:, d0:d0 + DPG]),
            "wv": arr_w(w_kv[:, EMB + d0:EMB + d0 + DPG]),
            "wo": arr_wo(w_o[d0:d0 + DPG, :]),
            "bo": (np.ascontiguousarray(b_o.reshape(1, EMB).astype(bf))
                   if g == 0 else zeros_bo),
            "cosT": cos_t,
            "sinM": sinm,
            "maskT": mask_t,
        })
    return in_maps


def gather_out(results):
    """Sum the 4 per-group bf16 partials per batch into the full f32 output."""
    parts = [np.asarray(results[c]["out"], np.float32) for c in range(N_CORES)]
    return np.stack([parts[0] + parts[1] + parts[2] + parts[3],
                     parts[4] + parts[5] + parts[6] + parts[7]], axis=0)


def kernel(x, w_kv, w_q, w_o, b_o):
    from concourse.bass_utils import run_bass_kernel_spmd

    nc = _get_nc(SEQ, has_bias=bool(np.any(np.asarray(b_o))))
    in_maps = make_in_maps(x, w_kv, w_q, w_o, b_o, SEQ)
    res = run_bass_kernel_spmd(nc, in_maps, core_ids=list(range(N_CORES)))
    return gather_out(res.results).astype(np.float32)


# revision 37
# speedup vs baseline: 1.2758x; 1.2758x over previous
"""Multi-head attention (QKV proj + RoPE + causal SDPA + out proj) on 8 TRN2 cores.

Sharding: core c = 4*b + g handles batch b (of 2) and head-group g (of 4, i.e.
4 heads = 512 feature dims). Per-core device kernel computes, for its batch:
    q/k/v projections for its 4 heads (column-sharded w_q / w_kv),
    RoPE on q and k, causal softmax attention,
    partial output projection with its 512 rows of w_o (+ bias on g==0 cores).
Host gathers by summing the 4 per-group partials per batch (the "all-reduce"
for the row-sharded w_o, done as the unshard step).

All inputs are cast to bf16 on the host so DMAs land directly in their final
SBUF tiles (no staging copies / on-device casts). Output is bf16, upcast and
summed on the host.

Device dataflow is feature-major: the host passes x pre-transposed (xT[e, s]);
projections produce qT/kT in [head_dim, seq] layout and v in [seq, head_dim]
layout, which is exactly what the S^T = K.Q^T and O^T = V^T.P^T matmuls need.

Engine balance: PE does only matmuls; ACT does the softmax exp (and the
PSUM->SBUF copies feeding RoPE / v, emitted during projection rounds where it
is otherwise idle); DVE does RoPE arithmetic, the P-accumulation that replaces
per-block rowsum matmuls, and PSUM evacuation. The rowsum matmul uses an
all-ones 128x128 stationary so its result lands already broadcast across
partitions (no gpsimd partition_broadcast, whose SWDGE DMA contends with DVE
for SBUF ports). Because exp throughput (1.2 G elem/s/lane) is slightly below
the 2-matmul PE pace, independent matmul chains (next chunk's q/k projections,
then the output projection) are interleaved into the attention instruction
stream so the PE never waits on ACT.
"""

import sys

import numpy as np

sys.path.insert(0, "/opt/trn_rl_repo")

EMB = 2048
SEQ = 2048
N_HEAD = 16
HD = 128
BATCH = 2
N_CORES = 8
GROUPS = 4  # head groups (tensor-parallel dimension)
HPG = N_HEAD // GROUPS  # heads per group = 4
DPG = HPG * HD  # feature dims per group = 512
NE = EMB // 128  # 16 e-blocks
SCALE = float(HD) ** -0.5


def _host_tables(seq):
    """cos / sign-folded sin RoPE tables in [d, s] layout + triangle mask."""
    d = HD
    inv = 1.0 / (10000.0 ** (np.arange(0, d, 2, dtype=np.float64) / d))  # [64]
    pos = np.arange(seq, dtype=np.float64)[None, :] * inv[:, None]  # [64, s]
    ang = np.concatenate([pos, pos], axis=0)  # [128, s]
    cos_t = np.cos(ang)
    sin = np.sin(ang)
    # rot is built as a plain partition swap (rot[0:64]=q[64:128], rot[64:128]=q[0:64]);
    # the rotate_half sign lives in the sin table instead.
    sinm = np.concatenate([-sin[:64], sin[64:]], axis=0)
    # triangle mask for the diagonal 128x128 block: keep (ko, qo) iff qo >= ko
    ko = np.arange(128)[:, None]
    qo = np.arange(128)[None, :]
    mask_t = (qo >= ko).astype(np.float64)
    return cos_t, sinm, mask_t


def build(seq=SEQ, has_bias=False):
    """Build the per-core Bass program. Returns the compiled Bacc module."""
    import concourse.bacc as bacc
    import concourse.tile as tile
    from concourse import mybir

    bf16 = mybir.dt.bfloat16

    assert seq % 512 == 0

    nc = bacc.Bacc("TRN2", target_bir_lowering=False, debug=False,
                   num_devices=N_CORES, name="mha8")

    # host pre-arranges x / weights so every DMA lands per-partition
    # contiguous (128 descriptors x 16KB instead of 1024 x 1KB)
    nj_ = seq // 512
    xt_d = nc.dram_tensor("xt", [nj_ * 128, NE * 512], bf16,
                          kind="ExternalInput")
    wq_d = nc.dram_tensor("wq", [128, NE * DPG], bf16, kind="ExternalInput")
    wk_d = nc.dram_tensor("wk", [128, NE * DPG], bf16, kind="ExternalInput")
    wv_d = nc.dram_tensor("wv", [128, NE * DPG], bf16, kind="ExternalInput")
    wo_d = nc.dram_tensor("wo", [128, HPG * EMB], bf16, kind="ExternalInput")
    bo_d = nc.dram_tensor("bo", [1, EMB], bf16, kind="ExternalInput")
    cos_d = nc.dram_tensor("cosT", [HD, seq], bf16, kind="ExternalInput")
    sinm_d = nc.dram_tensor("sinM", [HD, seq], bf16, kind="ExternalInput")
    mask_d = nc.dram_tensor("maskT", [128, 128], bf16, kind="ExternalInput")
    out_d = nc.dram_tensor("out", [seq, EMB], bf16, kind="ExternalOutput")

    with tile.TileContext(nc) as tc:
        _emit(nc, tc, tile, mybir, seq,
              xt_d, wq_d, wk_d, wv_d, wo_d, bo_d, cos_d, sinm_d, mask_d, out_d,
              has_bias)
    nc.compile()
    return nc


def _emit(nc, tc, tile, mybir, seq,
          xt_d, wq_d, wk_d, wv_d, wo_d, bo_d, cos_d, sinm_d, mask_d, out_d,
          has_bias):
    from contextlib import ExitStack

    f32 = mybir.dt.float32
    bf16 = mybir.dt.bfloat16
    EXP = mybir.ActivationFunctionType.Exp
    COPY = mybir.ActivationFunctionType.Copy
    nj = seq // 512  # 512-wide q chunks
    nsb = seq // 128

    ctx = ExitStack()
    with ctx:
        persist = ctx.enter_context(tc.tile_pool(name="persist", bufs=1))
        wpool = ctx.enter_context(tc.tile_pool(name="wpool", bufs=1, side="right"))

        # ---- constants / persistent tiles ----
        # all-ones stationary: rowsum matmul ones_mat.T @ acc yields the
        # rowsum already replicated across all 128 partitions
        ones_mat = persist.tile([128, 128], bf16, name="ones_mat")
        nc.vector.memset(ones_mat, 1.0)
        dummy = persist.tile([1, 1], f32, name="dummy")
        nc.vector.memset(dummy, 0.0)
        # pre-trigger the exp ACT table load so it overlaps the input DMAs
        nc.scalar.activation(dummy, dummy, EXP)

        mask_sb = persist.tile([128, 128], bf16, name="mask_sb")
        cos_sb = wpool.tile([128, seq], bf16, name="cos_sb")
        sinm_sb = wpool.tile([128, seq], bf16, name="sinm_sb")

        # per-(head, chunk) tensors: Tile's range tracking within a tile is
        # conservative, so readers of one chunk/head would falsely serialize
        # on writers of another; separate tiles make the independence exact
        kt = [[persist.tile([128, 512], bf16, name=f"kt{h}_{jj}")
               for jj in range(nj)] for h in range(HPG)]
        yt = [[persist.tile([128, 512], bf16, name=f"yt{h}_{jj}")
               for jj in range(nj)] for h in range(HPG)]
        v_sb = [persist.tile([128, 4, DPG], bf16, name=f"v_sb{jj}")
                for jj in range(nj)]  # [s_in, sb, d] per round

        w_sb = {nm: wpool.tile([128, NE, DPG], bf16, name=f"{nm}_sb")
                for nm in ("wq", "wk", "wv")}
        wo_sb = wpool.tile([128, HPG, EMB], bf16, name="wo_sb")

        xt_pool = ctx.enter_context(tc.tile_pool(name="xt", bufs=2, side="right"))
        qtj_pool = ctx.enter_context(tc.tile_pool(name="qtj", bufs=2))
        rope_pool = ctx.enter_context(tc.tile_pool(name="rope", bufs=2))
        pt_pool = ctx.enter_context(tc.tile_pool(name="pt", bufs=4))
        acc_pool = ctx.enter_context(tc.tile_pool(name="accp", bufs=2))
        sm_pool = ctx.enter_context(tc.tile_pool(name="sm", bufs=2))
        ob_pool = ctx.enter_context(tc.tile_pool(name="obp", bufs=2))

        ps_attn = ExitStack()
        st_pool = ps_attn.enter_context(
            tc.tile_pool(name="stp", bufs=3, space="PSUM"))
        ot_pool = ps_attn.enter_context(
            tc.tile_pool(name="otp", bufs=2, space="PSUM"))
        rs_pool = ps_attn.enter_context(
            tc.tile_pool(name="rsp", bufs=1, space="PSUM"))
        ps1_ctx = ExitStack()
        ps1 = ps1_ctx.enter_context(tc.tile_pool(name="ps1", bufs=2, space="PSUM"))

        # ---- DMA helpers (all direct bf16, no staging) ----
        # chunks: list of (e_count, engine). Finer chunks -> finer completion
        # semaphores, so consumers start on the first e-blocks sooner.
        def _chunks(spec, engs):
            if spec is not None:
                return spec
            n = len(engs)
            return [(NE // n, e) for e in engs]

        def load_w(nm, wd, engs=(None, None), chunks=None):
            e0 = 0
            for cnt, eng in _chunks(chunks, engs):
                src = wd[:, e0 * DPG:(e0 + cnt) * DPG].rearrange(
                    "p (e d) -> p e d", e=cnt)
                (eng or nc.sync).dma_start(w_sb[nm][:, e0:e0 + cnt, :], src)
                e0 += cnt

        def load_xt(j, engs=(None, None), chunks=None):
            xt_j = xt_pool.tile([128, NE, 512], bf16, name=f"xt_{j}", tag="xt")
            rows = xt_d[j * 128:(j + 1) * 128, :]
            e0 = 0
            for cnt, eng in _chunks(chunks, engs):
                src = rows[:, e0 * 512:(e0 + cnt) * 512].rearrange(
                    "p (e s) -> p e s", e=cnt)
                (eng or nc.sync).dma_start(xt_j[:, e0:e0 + cnt, :], src)
                e0 += cnt
            return xt_j

        def load_wo():
            for half in range(2):
                src = wo_d[:, half * 2 * EMB:(half + 1) * 2 * EMB].rearrange(
                    "p (h e) -> p h e", h=2)
                eng = nc.scalar if half else nc.sync
                eng.dma_start(wo_sb[:, half * 2:(half + 1) * 2, :], src)

        # ---- compute helpers ----
        def rope(dst, h, j, pp, tag):
            """dst = rope(pp) (chunk j); pp is the f32 PSUM projection."""
            sl = slice(j * 512, (j + 1) * 512)
            # ACT evacuates PSUM (idle during projection work); DVE does the rest
            qs = rope_pool.tile([128, 512], bf16, name=f"qs_{tag}", tag="qs")
            nc.scalar.activation(qs, pp, COPY)
            rot = rope_pool.tile([128, 512], bf16, name=f"rot_{tag}", tag="rot")
            # rotate_half as partition-shifted copies (sign folded into sinM);
            # TensorTensor ops must be partition-aligned, plain copies may shift
            nc.vector.tensor_copy(rot[0:64, :], qs[64:128, :])
            nc.vector.tensor_copy(rot[64:128, :], qs[0:64, :])
            nc.vector.tensor_mul(rot, rot, sinm_sb[:, sl])   # in-place
            nc.vector.tensor_mul(qs, qs, cos_sb[:, sl])      # in-place
            nc.vector.tensor_add(dst, qs, rot)

        qt_tiles = {}

        def qk_chain(j, h, nm, xt_j):
            """One 16-matmul projection chain + rope for (chunk j, head h)."""
            if j not in qt_tiles:
                qt_tiles[j] = qtj_pool.tile([128, HPG, 512], bf16,
                                            name=f"qt_{j}", tag="qtj")
            pp = ps1.tile([128, 512], f32, name=f"pp_{nm}_{h}_{j}", tag="proj")
            for e in range(NE):
                nc.tensor.matmul(pp, w_sb[nm][:, e, h * 128:(h + 1) * 128],
                                 xt_j[:, e, :], start=(e == 0), stop=(e == NE - 1))
            if nm == "wq":
                rope(qt_tiles[j][:, h, :], h, j, pp, f"q{h}_{j}")
            else:
                rope(kt[h][j], h, j, pp, f"k{h}_{j}")

        def v_chain(j, sb, xt_j):
            i_blk = j * 4 + sb
            vp = ps1.tile([128, DPG], f32, name=f"vp_{i_blk}", tag="proj")
            for e in range(NE):
                nc.tensor.matmul(vp, xt_j[:, e, sb * 128:(sb + 1) * 128],
                                 w_sb["wv"][:, e, :], start=(e == 0),
                                 stop=(e == NE - 1))
            nc.scalar.activation(v_sb[j][:, sb, :], vp, COPY)

        bo_sb = ones_row = None

        def op_block(sb, pool, out_eng, cast_eng=None, split_dma=False):
            """Output projection for seq row-block sb: [128, 2048] partial."""
            ssl = slice(sb * 128, (sb + 1) * 128)
            ob = ob_pool.tile([128, EMB], bf16, name=f"ob_{sb}", tag="ob")
            for ec in range(EMB // 512):
                esl = slice(ec * 512, (ec + 1) * 512)
                op = pool.tile([128, 512], f32, name=f"op_{sb}_{ec}", tag="proj")
                for h in range(HPG):
                    nc.tensor.matmul(op, yt[h][sb // 4][:, (sb % 4) * 128:(sb % 4 + 1) * 128], wo_sb[:, h, esl],
                                     start=(h == 0),
                                     stop=(not has_bias and h == HPG - 1))
                if has_bias:
                    nc.tensor.matmul(op, ones_row, bo_sb[:, esl],
                                     start=False, stop=True)
                if cast_eng is nc.scalar:
                    nc.scalar.copy(ob[:, esl], op)
                else:
                    nc.vector.tensor_copy(ob[:, esl], op)
                if split_dma and ec % 2:  # drain each half as soon as cast
                    out_eng.dma_start(out_d[ssl, (ec - 1) * 512:(ec + 1) * 512],
                                      ob[:, (ec - 1) * 512:(ec + 1) * 512])
            # during attention the scalar/ACT ring must stay free for exps (a
            # waiting DMA at the ACT queue head would block them) -> sync only
            if not split_dma:
                out_eng.dma_start(out_d[ssl, :], ob)

        # ---- preamble DMAs ----
        # wq on the scalar ring / xt0 on the sync ring, leading chunks tiny so
        # the first projection chain starts on e-blocks 0-1 ASAP; tables
        # follow, then wk, wv.
        load_w("wq", wq_d, chunks=[(4, nc.scalar)] * 4)
        xt_cur = load_xt(0, chunks=[(4, nc.sync)] * 4)
        nc.sync.dma_start(cos_sb, cos_d[:])
        nc.sync.dma_start(sinm_sb, sinm_d[:])
        nc.scalar.dma_start(mask_sb, mask_d[:])
        load_w("wk", wk_d, (nc.scalar, nc.sync))
        load_w("wv", wv_d, (nc.scalar, nc.sync))
        if has_bias:
            ones_row = persist.tile([1, 128], bf16, name="ones_row")
            nc.vector.memset(ones_row, 1.0)
            bo_sb = persist.tile([1, EMB], bf16, name="bo_sb")
            nc.scalar.dma_start(bo_sb, bo_d[:])

        # round 0 projections run before any attention exists to interleave;
        # q chains first (wq arrives before wk)
        for nm in ("wq", "wk"):
            for h in range(HPG):
                qk_chain(0, h, nm, xt_cur)

        xt_next = load_xt(1)
        load_wo()

        # ---- main rounds ----
        for j in range(nj):
            for sb in range(4):
                v_chain(j, sb, xt_cur)

            if j == nj - 1:
                # projections done: free ps1's banks and reopen them for the
                # interleaved output projection
                ps1_ctx.close()
                ps3_ctx = ExitStack()
                ps3 = ps3_ctx.enter_context(tc.tile_pool(name="ps3", bufs=2,
                                                         space="PSUM"))

            # filler queue: independent PE chains interleaved into attention
            # so the PE keeps running while ACT works through the exps
            fillers = []
            if j + 1 < nj:
                xt_for_next = xt_next
                for h in range(HPG):
                    for nm in ("wq", "wk"):
                        fillers.append((lambda jj=j + 1, hh=h, nn=nm,
                                        xx=xt_for_next:
                                        qk_chain(jj, hh, nn, xx)))
            else:
                for sb in range(12):
                    fillers.append(lambda s=sb: op_block(s, ps3, nc.sync))
            fill_idx = 0
            # fillers per head, always one at the head's LAST block (the
            # S->exp->PV drain there otherwise idles the PE ~1us), rest spread
            fph = max(1, len(fillers) // HPG)
            nblk = 4 * j + 4
            fill_pos = set()
            for t in range(fph):
                fill_pos.add(nblk - 1 if t == fph - 1
                             else ((t + 1) * nblk) // fph - 1)

            def make_norm(h, acc, ot, pt_last):
                def norm():
                    # rowsum + partition-broadcast via the all-ones stationary,
                    # in two matmuls: acc (blocks 0..n-2, ready early) plus the
                    # final block's masked pt directly (skips its DVE acc-add,
                    # which otherwise sits queued behind filler casts)
                    rb = rs_pool.tile([128, 512], f32, name=f"rb_{h}_{j}",
                                      tag="rs")
                    nc.tensor.matmul(rb, ones_mat, acc, start=True, stop=False)
                    nc.tensor.matmul(rb[:, 384:512], ones_mat, pt_last[:, 0:128],
                                     start=False, stop=True)
                    rbf = sm_pool.tile([128, 512], f32, name=f"rbf_{h}_{j}",
                                       tag="rbf")
                    nc.vector.reciprocal_approx_fast(rbf, rb)
                    nc.vector.tensor_mul(yt[h][j], ot, rbf)
                return norm

            pending_norm = None
            for h in range(HPG):
                ot = ot_pool.tile([128, 512], f32, name=f"ot_{h}_{j}", tag="ot")
                acc = acc_pool.tile([128, 512], bf16, name=f"acc_{h}_{j}",
                                    tag="acc")
                for i in range(nblk):
                    m = i - 4 * j  # diagonal index (>=0 on the 4 trailing blocks)
                    qoff = max(m, 0) * 128
                    n = 512 - qoff
                    st = st_pool.tile([128, 512], f32, name=f"st_{h}_{j}_{i}",
                                      tag="st", bufs=3)
                    nc.tensor.matmul(
                        st[:, 0:n], kt[h][i // 4][:, (i % 4) * 128:
                                                  (i % 4 + 1) * 128],
                        qt_tiles[j][:, h, qoff:512], start=True, stop=True)
                    pt = pt_pool.tile([128, 512], bf16, name=f"pt_{h}_{j}_{i}",
                                      tag="pt")
                    nc.scalar.activation(pt[:, 0:n], st[:, 0:n], EXP, scale=SCALE)
                    # previous head's normalize slots in at block 1: its
                    # rowsum needs the prior head's last exp+acc (~1.1us after
                    # its last S), and by block 1 the PE has enough queued
                    # work to cover that latency
                    if pending_norm is not None and i == 1:
                        pending_norm()
                        pending_norm = None
                    # filler sits between this block's S and PV: the PE chews
                    # through it while ACT finishes the exp, so PV never waits
                    if i in fill_pos and fill_idx < len(fillers):
                        fillers[fill_idx]()
                        fill_idx += 1
                    if m >= 0:  # triangle mask on the leading 128 valid q cols
                        nc.vector.tensor_mul(pt[:, 0:128], pt[:, 0:128], mask_sb)
                    if i == 0:
                        nc.vector.tensor_copy(acc, pt)
                    elif i < nblk - 1:
                        nc.vector.tensor_add(acc[:, qoff:512], acc[:, qoff:512],
                                             pt[:, 0:n])
                    else:
                        pt_last = pt
                    nc.tensor.matmul(ot[:, qoff:512],
                                     v_sb[i // 4][:, i % 4,
                                                  h * 128:(h + 1) * 128],
                                     pt[:, 0:n],
                                     start=(i == 0), stop=(i == nblk - 1))
                pending_norm = make_norm(h, acc, ot, pt_last)
            pending_norm()
            while fill_idx < len(fillers):
                fillers[fill_idx]()
                fill_idx += 1

            xt_cur = xt_next
            if j + 2 < nj:
                xt_next = load_xt(j + 2)

        # remaining output-projection row blocks (need round-3 attention);
        # keep ps3 — swapping pools here would barrier them behind the whole
        # final normalize chain instead of just their h3 matmuls
        for sb in range(12, nsb):
            op_block(sb, ps3, nc.scalar if sb % 2 else nc.sync,
                     cast_eng=nc.scalar, split_dma=True)
        ps3_ctx.close()
        ps_attn.close()


_NC_CACHE = {}


def _get_nc(seq=SEQ, has_bias=False):
    key = (seq, has_bias)
    if key not in _NC_CACHE:
        _NC_CACHE[key] = build(seq, has_bias)
    return _NC_CACHE[key]


def make_in_maps(x, w_kv, w_q, w_o, b_o, seq=SEQ):
    """Shard full inputs into the 8 per-core input dicts (all bf16)."""
    import ml_dtypes

    bf = ml_dtypes.bfloat16
    cos_t, sinm, mask_t = _host_tables(seq)
    cos_t = np.ascontiguousarray(cos_t.astype(bf))
    sinm = np.ascontiguousarray(sinm.astype(bf))
    mask_t = np.ascontiguousarray(mask_t.astype(bf))
    zeros_bo = np.zeros((1, EMB), bf)
    x = np.asarray(x, np.float32)
    w_kv = np.asarray(w_kv, np.float32)
    w_q = np.asarray(w_q, np.float32)
    w_o = np.asarray(w_o, np.float32)
    b_o = np.asarray(b_o, np.float32)
    nj = seq // 512

    def arr_w(w):  # [2048, 512] -> [128, 16*512]: row p holds (e, d) runs
        return np.ascontiguousarray(
            w.reshape(NE, 128, DPG).transpose(1, 0, 2).reshape(128, NE * DPG)
            .astype(bf))

    def arr_wo(w):  # [512, 2048] -> [128, 4*2048]
        return np.ascontiguousarray(
            w.reshape(HPG, 128, EMB).transpose(1, 0, 2).reshape(128, HPG * EMB)
            .astype(bf))

    def arr_xt(xb):  # [seq, 2048] -> [nj*128, 16*512]: xT chunked + contiguous
        a = xb.reshape(nj, 512, NE, 128).transpose(0, 3, 2, 1)
        return np.ascontiguousarray(
            a.reshape(nj * 128, NE * 512).astype(bf))

    xts = [arr_xt(x[b]) for b in range(BATCH)]
    in_maps = []
    for c in range(N_CORES):
        b, g = divmod(c, GROUPS)
        d0 = g * DPG
        in_maps.append({
            "xt": xts[b],
            "wq": arr_w(w_q[:, d0:d0 + DPG]),
            "wk": arr_w(w_kv[:, d0:d0 + DPG]),
            "wv": arr_w(w_kv[:, EMB + d0:EMB + d0 + DPG]),
            "wo": arr_wo(w_o[d0:d0 + DPG, :]),
            "bo": (np.ascontiguousarray(b_o.reshape(1, EMB).astype(bf))
                   if g == 0 else zeros_bo),
            "cosT": cos_t,
            "sinM": sinm,
            "maskT": mask_t,
        })
    return in_maps


def gather_out(results):
    """Sum the 4 per-group bf16 partials per batch into the full f32 output."""
    parts = [np.asarray(results[c]["out"], np.float32) for c in range(N_CORES)]
    return np.stack([parts[0] + parts[1] + parts[2] + parts[3],
                     parts[4] + parts[5] + parts[6] + parts[7]], axis=0)


def kernel(x, w_kv, w_q, w_o, b_o):
    from concourse.bass_utils import run_bass_kernel_spmd

    nc = _get_nc(SEQ, has_bias=bool(np.any(np.asarray(b_o))))
    in_maps = make_in_maps(x, w_kv, w_q, w_o, b_o, SEQ)
    res = run_bass_kernel_spmd(nc, in_maps, core_ids=list(range(N_CORES)))
    return gather_out(res.results).astype(np.float32)
